# revision 6
# baseline (speedup 1.0000x reference)
"""ColBERT MaxSim kernel v4 for 8 Trainium2 NeuronCores (Bass/Tile).

Math (matches the reference):
  Q  = l2norm(q_hidden @ W^T)                       (64, 32, 128)
  D  = l2norm(d_hidden @ W^T), masked tokens zeroed (512, 256, 128)
  sim[b,n,q,d] = Q[b] @ D[b*8+n]^T ; masked -> 0 (true maxima > 0)
  out[b,n] = mean_q max_d sim                       (64, 8)

Sharding: data-parallel over the query-group dim B=64 -> 8 groups per
core; each core owns the matching 64 docs. W replicated. No cross-core
communication.

v4 design (measured ~58-62us vs v3's 87.4us):
 - Projection runs as fp8e4 x fp8e4 DoubleRow matmuls (2 k-tiles of
   contraction per instruction = 157 TF/s, the fp8 roofline; measured
   213ns per 512-token matmul with LDWEIGHTS fully hidden). W is scaled
   by 16 (exact power of 2, cancelled by the L2 normalization) to clear
   e4m3's subnormal floor before the host-side fp8 cast.
 - D is never materialized normalized: sim columns are scaled by
   inv_norm at the [32q x token] level instead, then max-reduced (DVE).
 - sim and s2 matmuls write 4 token-blocks into ONE [128,512] PSUM
   tile at 4 PE tile positions (32-col stationary tiles at col offsets
   0/32/64/96), so rsqrt + scale + max process 4 tokens per column:
   4x less ACT/DVE work on the normalize/max path.
 - Block-major projection order releases each block's PSUM->SBUF copy
   early; band matmuls of superblock g run after the projections of
   g+1 (software pipelining) so copy/square latency is hidden.
 - ~12 warmup matmuls at kernel start ramp the PE DVFS p-state to
   2.4GHz through the ~8us DMA/engine-init preamble, and the first
   superblock's DMA is issued ahead of the small W/query streams.
Engines per 2048-token superblock (steady state, measured): PE 4.26us
(pacer: 12 DR proj + 4 sim + 4 s2 matmuls), DMA 3.93us @ 400GB/s,
ACT ~3.4us, DVE ~3.4us. Steady-state period 4.37us/superblock.
"""

import sys

sys.path.insert(0, "/opt/trn_rl_repo")

from contextlib import ExitStack

import ml_dtypes
import numpy as np

import concourse.bass as bass
import concourse.tile as tile
from concourse import bacc, mybir
from concourse.bass import ts, ds
from concourse.bass_utils import run_bass_kernel_spmd

B_Q, L_Q = 64, 32
B_D, L_D = 512, 256
HID, OUT = 768, 128
N_CORES = 8

GROUPS = B_Q // N_CORES            # 8 query groups (superblocks) per core
N_P = B_D // B_Q                   # 8 docs per group
DTOK = GROUPS * N_P * L_D          # 16384 doc tokens per core
QTOK = GROUPS * L_Q                # 256 query tokens per core
K_CH = HID // 128                  # 6 contraction chunks
TN = 512                           # doc tokens per block (= matmul width)
BLOCKS = 4                         # blocks per superblock
SBTOK = BLOCKS * TN                # 2048 tokens per superblock = 1 group
D_TILES = DTOK // TN               # 32
W_SCALE = 16.0                     # power of 2; cancelled by l2norm
F32 = mybir.dt.float32
BF16 = mybir.dt.bfloat16
FP8 = mybir.dt.float8e4
AFT = mybir.ActivationFunctionType
DR = mybir.MatmulPerfMode.DoubleRow


def _build_program(trace_sim=False, proj_order="block", bands="pipelined",
                   psp=3, pss=1, dsq_pool=0, dma_split=1, warmup=16,
                   pair_tiles=False, dx_queue="sync"):
    nc = bacc.Bacc("TRN2", target_bir_lowering=False, debug=False,
                   num_devices=N_CORES)

    # host layouts: one doc tile = [128 part, 6 kchunk, 512 tok] contiguous
    dT = nc.dram_tensor("dT", [D_TILES, 128, K_CH // 2, 2 * TN], FP8,
                        kind="ExternalInput").ap()
    qT = nc.dram_tensor("qT", [128, K_CH, QTOK], BF16,
                        kind="ExternalInput").ap()
    wT = nc.dram_tensor("wT", [128, K_CH, OUT], BF16,
                        kind="ExternalInput").ap()
    w8T = nc.dram_tensor("w8T", [128, K_CH, OUT], FP8,
                         kind="ExternalInput").ap()
    out = nc.dram_tensor("out", [BLOCKS, GROUPS * 2], F32,
                         kind="ExternalOutput").ap()

    with tile.TileContext(nc, trace_sim=trace_sim) as tc, ExitStack() as ctx:
        const = ctx.enter_context(tc.tile_pool(name="const", bufs=1))
        persist = ctx.enter_context(tc.tile_pool(name="persist", bufs=1))
        sbx = ctx.enter_context(tc.tile_pool(name="sbx", bufs=3))
        sbc = ctx.enter_context(tc.tile_pool(name="sbc", bufs=(5 if pair_tiles else 9)))
        sbq = ctx.enter_context(tc.tile_pool(name="sbq", bufs=(5 if pair_tiles else 9)))
        sbi = ctx.enter_context(tc.tile_pool(name="sbi", bufs=2))
        sbs = ctx.enter_context(tc.tile_pool(name="sbs", bufs=2))
        qsb = ctx.enter_context(tc.tile_pool(name="qsb", bufs=1))

        w8 = const.tile([128, K_CH, OUT], FP8)
        nc.sync.dma_start(out=w8[:], in_=w8T[:, :, :])
        wt = const.tile([128, K_CH, OUT], BF16)
        ones128 = const.tile([128, 128], BF16)
        nc.vector.memset(ones128[:], 1.0)
        wsrc = const.tile([128, TN], BF16)
        nc.vector.memset(wsrc[:], 1.0)
        eps128 = const.tile([128, 1], F32)
        nc.vector.memset(eps128[:], 1.0e-4)
        band_ones = const.tile([128, BLOCKS], F32)
        nc.vector.memset(band_ones[:], 0.0)
        for j in range(BLOCKS):
            nc.vector.memset(band_ones[32 * j:32 * (j + 1), j:j + 1], 1.0)

        Qn = persist.tile([128, QTOK], BF16)      # normalized query embeds
        mx = persist.tile([128, GROUPS * 2], F32)  # packed per-doc maxima
        out_sb = persist.tile([BLOCKS, GROUPS * 2], F32)

        with (
            tc.tile_pool(name="psD", bufs=(2 if pair_tiles else 4), space="PSUM") as psD,
            tc.tile_pool(name="psP", bufs=psp, space="PSUM") as psP,
            tc.tile_pool(name="psS", bufs=pss, space="PSUM") as psS,
        ):
            # ---- PE p-state warmup: ~8 dummy matmuls keep the tensor
            # engine continuously busy through the DMA preamble so it ramps
            # to 2.4GHz before the first real projection. ----
            if warmup:
                wrm = psP.tile([128, TN], F32, tag="sim", name="wrm")
                for u in range(warmup):
                    nc.tensor.matmul(wrm[:], ones128[:], wsrc[:],
                                     start=(u == 0), stop=(u == warmup - 1),
                                     skip_group_check=True)

            # ---- startup DMA order: first dx superblock ahead of the
            # query/bf16-W streams so the projection pipeline fills ASAP ----
            dx0 = sbx.tile([128, BLOCKS, K_CH // 2, 2 * TN], FP8, tag="dx")
            for hh in range(2):
                nc.sync.dma_start(
                    out=dx0[:, ds(2 * hh, 2)],
                    in_=dT[ds(2 * hh, 2)].rearrange("a p k t -> p a k t"))
            nc.sync.dma_start(out=wt[:], in_=wT[:, :, :])

            # ---- query phase: project + L2-normalize 256 query tokens ----
            qx = qsb.tile([128, K_CH, QTOK], BF16, tag="qx")
            nc.sync.dma_start(out=qx[:], in_=qT[:, :, :])
            qt_ps = psP.tile([128, QTOK], F32, tag="sim")
            for k in range(K_CH):
                nc.tensor.matmul(qt_ps[:], wt[:, k, :], qx[:, k, :],
                                 start=(k == 0), stop=(k == K_CH - 1))
            qt_sb = qsb.tile([128, QTOK], BF16, tag="qtc")
            nc.scalar.activation(qt_sb[:], qt_ps[:], AFT.Copy)
            qsq = qsb.tile([128, QTOK], BF16, tag="qsq")
            nc.vector.tensor_mul(qsq[:], qt_sb[:], qt_sb[:])
            qs2 = psS.tile([128, QTOK], F32, tag="s2")
            nc.tensor.matmul(qs2[:], ones128[:], qsq[:], start=True, stop=True)
            qinv = qsb.tile([128, QTOK], BF16, tag="qinv")
            nc.scalar.activation(qinv[:], qs2[:], AFT.Abs_reciprocal_sqrt)
            nc.vector.tensor_mul(Qn[:], qt_sb[:], qinv[:])

            # ---- doc loop: 8 superblocks of 2048 tokens (1 group) ----
            def emit_proj(g):
                if g == 0:
                    dx = dx0
                else:
                    dx = sbx.tile([128, BLOCKS, K_CH // 2, 2 * TN], FP8,
                                  tag="dx")
                    dma_eng = nc.gpsimd if dx_queue == "gpsimd" else nc.sync
                    for hh in range(dma_split):
                        w_ = BLOCKS // dma_split
                        dma_eng.dma_start(
                            out=dx[:, ds(w_ * hh, w_)],
                            in_=dT[ds(BLOCKS * g + w_ * hh, w_)].rearrange(
                                "a p k t -> p a k t"))
                dts = []
                if pair_tiles:
                    for h in range(2):
                        dt_ps = psD.tile([128, 2 * TN], F32, tag="dt",
                                         name=f"dtp{h}")
                        for jj in range(2):
                            j = 2 * h + jj
                            for kk in range(K_CH // 2):
                                nc.tensor.matmul(
                                    dt_ps[:, ds(jj * TN, TN)],
                                    w8[:, ds(2 * kk, 2), :],
                                    dx[:, j, ds(2 * kk, 2), :],
                                    start=(kk == 0),
                                    stop=(kk == K_CH // 2 - 1),
                                    perf_mode=DR)
                        dt_sb = sbc.tile([128, 2 * TN], BF16, tag="dtc")
                        nc.scalar.activation(dt_sb[:], dt_ps[:], AFT.Copy)
                        dsq = sbq.tile([128, 2 * TN], BF16, tag="dsq")
                        eng = nc.gpsimd if h < dsq_pool else nc.vector
                        eng.tensor_mul(dsq[:], dt_sb[:], dt_sb[:])
                        dts.append((dt_sb[:, 0:TN], dsq[:, 0:TN]))
                        dts.append((dt_sb[:, ds(TN, TN)], dsq[:, ds(TN, TN)]))
                else:
                    dt_ps = [psD.tile([128, TN], F32, tag="dt", name=f"dt{j}")
                             for j in range(BLOCKS)]
                    for j in range(BLOCKS):
                        for kk in range(K_CH // 2):
                            nc.tensor.matmul(
                                dt_ps[j][:], w8[:, ds(2 * kk, 2), :],
                                dx[:, j, kk, :].rearrange(
                                    "p (t k) -> p k t", k=2),
                                start=(kk == 0), stop=(kk == K_CH // 2 - 1),
                                perf_mode=DR)
                    for j in range(BLOCKS):
                        dt_sb = sbc.tile([128, TN], BF16, tag="dtc")
                        nc.scalar.activation(dt_sb[:], dt_ps[j][:], AFT.Copy)
                        dsq = sbq.tile([128, TN], BF16, tag="dsq")
                        eng = nc.gpsimd if j < dsq_pool else nc.vector
                        eng.tensor_mul(dsq[:], dt_sb[:], dt_sb[:])
                        dts.append((dt_sb[:], dsq[:]))
                return dts

            def emit_bands(g, dts):
                sim = psP.tile([128, TN], F32, tag="sim")
                s2p = psS.tile([128, TN], F32, tag="s2")
                for i, (dt_sb, dsq) in enumerate(dts):
                    nc.tensor.matmul(sim[ds(32 * i, 32), :],
                                     Qn[:, ts(g, L_Q)], dt_sb,
                                     start=True, stop=True,
                                     tile_position=(0, 32 * i))
                for i, (dt_sb, dsq) in enumerate(dts):
                    nc.tensor.matmul(s2p[ds(32 * i, 32), :],
                                     ones128[:, 0:32], dsq,
                                     start=True, stop=True,
                                     tile_position=(0, 32 * i))
                inv = sbi.tile([128, TN], BF16, tag="inv")
                nc.scalar.activation(inv[:], s2p[:], AFT.Abs_reciprocal_sqrt,
                                     bias=eps128[:])
                scr = sbs.tile([128, TN], BF16, tag="scr")
                nc.vector.tensor_mul(scr[:], sim[:], inv[:])
                nc.vector.tensor_reduce(
                    mx[:, ds(2 * g, 2)],
                    scr[:].rearrange("p (n d) -> p n d", n=2),
                    axis=mybir.AxisListType.X, op=mybir.AluOpType.max)

            if bands == "insection":
                for g in range(GROUPS):
                    dts = emit_proj(g)
                    emit_bands(g, dts)
            else:
                prev = None
                for g in range(GROUPS):
                    dts = emit_proj(g)
                    if prev is not None:
                        emit_bands(*prev)
                    prev = (g, dts)
                emit_bands(*prev)

            # ---- mean over the 32 queries of each band (via matmul) ----
            mean_ps = psS.tile([BLOCKS, GROUPS * 2], F32, tag="s2")
            nc.tensor.matmul(mean_ps[:], band_ones[:], mx[:],
                             start=True, stop=True)
            nc.vector.tensor_scalar_mul(out_sb[:], mean_ps[:], 1.0 / L_Q)
        nc.sync.dma_start(out=out[:, :], in_=out_sb[:])

    nc.compile()
    return nc


def _shard_inputs(q_hidden, d_hidden, d_input_ids, skiplist, W):
    """Host-side shard + relayout + dtype cast. Returns per-core in_maps."""
    q_hidden = np.asarray(q_hidden, dtype=np.float32)
    d_hidden = np.asarray(d_hidden, dtype=np.float32)
    ids = np.asarray(d_input_ids)
    skip = np.asarray(skiplist)

    # cast first: quarters the bytes the host transposes afterwards
    dh8 = d_hidden.astype(ml_dtypes.float8_e4m3)
    qh16 = q_hidden.astype(ml_dtypes.bfloat16)
    w_t = np.asarray(W, dtype=np.float32).T                   # [768, 128]
    w16 = w_t.astype(ml_dtypes.bfloat16)
    w8 = (w_t * W_SCALE).astype(ml_dtypes.float8_e4m3)

    # zero masked tokens: projections/norms become exactly 0 on device and
    # the biased rsqrt keeps inv finite, so their sims are exactly 0
    masked = (ids == 0) | np.isin(ids, skip)
    dh8[masked] = 0

    wH = np.ascontiguousarray(
        w16.reshape(K_CH, 128, OUT).transpose(1, 0, 2))       # [128, 6, 128]
    w8H = np.ascontiguousarray(
        w8.reshape(K_CH, 128, OUT).transpose(1, 0, 2))
    in_maps = []
    for c in range(N_CORES):
        dh = dh8[c * 64:(c + 1) * 64].reshape(-1, HID)        # [16384, 768]
        qh = qh16[c * GROUPS:(c + 1) * GROUPS].reshape(-1, HID)
        dH = np.ascontiguousarray(
            dh.reshape(D_TILES, TN, K_CH // 2, 2, 128)
            .transpose(0, 4, 2, 1, 3)
            .reshape(D_TILES, 128, K_CH // 2, 2 * TN))
        qH = np.ascontiguousarray(
            qh.reshape(QTOK, K_CH, 128).transpose(2, 1, 0))   # [128, 6, 256]
        in_maps.append({
            "dT": dH,                          # [32, 128, 6, 512] fp8e4m3
            "qT": qH,
            "wT": wH,
            "w8T": w8H,
        })
    return in_maps


_CACHED = {}


def _get_program(key=("default",), **kw):
    if key not in _CACHED:
        _CACHED[key] = _build_program(**kw)
    return _CACHED[key]


def _unpack_out(arr):
    """[4, 16] device tile -> [8 groups, 8 docs]."""
    return np.ascontiguousarray(
        arr.reshape(BLOCKS, GROUPS, 2).transpose(1, 0, 2).reshape(
            GROUPS, N_P))


def kernel(q_hidden, d_hidden, d_input_ids, skiplist, W):
    nc = _get_program(key=("ship",))
    in_maps = _shard_inputs(q_hidden, d_hidden, d_input_ids, skiplist, W)
    res = run_bass_kernel_spmd(nc, in_maps, list(range(N_CORES)))
    outs = [_unpack_out(res.results[c]["out"]) for c in range(N_CORES)]
    return np.concatenate(outs, axis=0)                # (64, 8)


# revision 7
# speedup vs baseline: 1.0425x; 1.0425x over previous
"""ColBERT MaxSim kernel v4 for 8 Trainium2 NeuronCores (Bass/Tile).

Math (matches the reference):
  Q  = l2norm(q_hidden @ W^T)                       (64, 32, 128)
  D  = l2norm(d_hidden @ W^T), masked tokens zeroed (512, 256, 128)
  sim[b,n,q,d] = Q[b] @ D[b*8+n]^T ; masked -> 0 (true maxima > 0)
  out[b,n] = mean_q max_d sim                       (64, 8)

Sharding: data-parallel over the query-group dim B=64 -> 8 groups per
core; each core owns the matching 64 docs. W replicated. No cross-core
communication.

v4 design (measured ~58-62us vs v3's 87.4us):
 - Projection runs as fp8e4 x fp8e4 DoubleRow matmuls (2 k-tiles of
   contraction per instruction = 157 TF/s, the fp8 roofline; measured
   213ns per 512-token matmul with LDWEIGHTS fully hidden). W is scaled
   by 16 (exact power of 2, cancelled by the L2 normalization) to clear
   e4m3's subnormal floor before the host-side fp8 cast.
 - D is never materialized normalized: sim columns are scaled by
   inv_norm at the [32q x token] level instead, then max-reduced (DVE).
 - sim and s2 matmuls write 4 token-blocks into ONE [128,512] PSUM
   tile at 4 PE tile positions (32-col stationary tiles at col offsets
   0/32/64/96), so rsqrt + scale + max process 4 tokens per column:
   4x less ACT/DVE work on the normalize/max path.
 - Block-major projection order releases each block's PSUM->SBUF copy
   early; band matmuls of superblock g run after the projections of
   g+1 (software pipelining) so copy/square latency is hidden.
 - 16 warmup matmuls at kernel start ramp the PE DVFS p-state to
   2.4GHz and keep it hot through the ~8us DMA/engine-init preamble
   until the first projection's data lands; the first superblock's DMA
   is issued ahead of the small W/query streams. The doc stream is
   stored with DoubleRow k-pairs byte-interleaved per token.
Engines per 2048-token superblock (steady state, measured): PE 4.26us
(pacer: 12 DR proj + 4 sim + 4 s2 matmuls), DMA 3.93us @ 400GB/s,
ACT ~3.4us, DVE ~3.4us. Steady-state period 4.37us/superblock.
"""

import sys

sys.path.insert(0, "/opt/trn_rl_repo")

from contextlib import ExitStack

import ml_dtypes
import numpy as np

import concourse.bass as bass
import concourse.tile as tile
from concourse import bacc, mybir
from concourse.bass import ts, ds
from concourse.bass_utils import run_bass_kernel_spmd

B_Q, L_Q = 64, 32
B_D, L_D = 512, 256
HID, OUT = 768, 128
N_CORES = 8

GROUPS = B_Q // N_CORES            # 8 query groups (superblocks) per core
N_P = B_D // B_Q                   # 8 docs per group
DTOK = GROUPS * N_P * L_D          # 16384 doc tokens per core
QTOK = GROUPS * L_Q                # 256 query tokens per core
K_CH = HID // 128                  # 6 contraction chunks
TN = 512                           # doc tokens per block (= matmul width)
BLOCKS = 4                         # blocks per superblock
SBTOK = BLOCKS * TN                # 2048 tokens per superblock = 1 group
D_TILES = DTOK // TN               # 32
W_SCALE = 16.0                     # power of 2; cancelled by l2norm
F32 = mybir.dt.float32
BF16 = mybir.dt.bfloat16
FP8 = mybir.dt.float8e4
AFT = mybir.ActivationFunctionType
DR = mybir.MatmulPerfMode.DoubleRow


def _build_program(trace_sim=False, proj_order="block", bands="pipelined",
                   psp=3, pss=1, dsq_pool=0, dma_split=1, warmup=16,
                   pair_tiles=False, dx_queue="sync"):
    nc = bacc.Bacc("TRN2", target_bir_lowering=False, debug=False,
                   num_devices=N_CORES)

    # host layouts: one doc tile = [128 part, 6 kchunk, 512 tok] contiguous
    dT = nc.dram_tensor("dT", [D_TILES, 128, K_CH // 2, 2 * TN], FP8,
                        kind="ExternalInput").ap()
    qT = nc.dram_tensor("qT", [128, K_CH, QTOK], BF16,
                        kind="ExternalInput").ap()
    wT = nc.dram_tensor("wT", [128, K_CH, OUT], BF16,
                        kind="ExternalInput").ap()
    w8T = nc.dram_tensor("w8T", [128, K_CH, OUT], FP8,
                         kind="ExternalInput").ap()
    out = nc.dram_tensor("out", [BLOCKS, GROUPS * 2], F32,
                         kind="ExternalOutput").ap()

    with tile.TileContext(nc, trace_sim=trace_sim) as tc, ExitStack() as ctx:
        const = ctx.enter_context(tc.tile_pool(name="const", bufs=1))
        persist = ctx.enter_context(tc.tile_pool(name="persist", bufs=1))
        sbx = ctx.enter_context(tc.tile_pool(name="sbx", bufs=3))
        sbc = ctx.enter_context(tc.tile_pool(name="sbc", bufs=(5 if pair_tiles else 9)))
        sbq = ctx.enter_context(tc.tile_pool(name="sbq", bufs=(5 if pair_tiles else 9)))
        sbi = ctx.enter_context(tc.tile_pool(name="sbi", bufs=2))
        sbs = ctx.enter_context(tc.tile_pool(name="sbs", bufs=2))
        qsb = ctx.enter_context(tc.tile_pool(name="qsb", bufs=1))

        w8 = const.tile([128, K_CH, OUT], FP8)
        nc.sync.dma_start(out=w8[:], in_=w8T[:, :, :])
        wt = const.tile([128, K_CH, OUT], BF16)
        ones128 = const.tile([128, 128], BF16)
        nc.vector.memset(ones128[:], 1.0)
        wsrc = const.tile([128, TN], BF16)
        nc.vector.memset(wsrc[:], 1.0)
        eps128 = const.tile([128, 1], F32)
        nc.vector.memset(eps128[:], 1.0e-4)
        band_ones = const.tile([128, BLOCKS], F32)
        nc.vector.memset(band_ones[:], 0.0)
        for j in range(BLOCKS):
            nc.vector.memset(band_ones[32 * j:32 * (j + 1), j:j + 1], 1.0)

        Qn = persist.tile([128, QTOK], BF16)      # normalized query embeds
        mx = persist.tile([128, GROUPS * 2], F32)  # packed per-doc maxima
        out_sb = persist.tile([BLOCKS, GROUPS * 2], F32)

        with (
            tc.tile_pool(name="psD", bufs=(2 if pair_tiles else 4), space="PSUM") as psD,
            tc.tile_pool(name="psP", bufs=psp, space="PSUM") as psP,
            tc.tile_pool(name="psS", bufs=pss, space="PSUM") as psS,
        ):
            # ---- PE p-state warmup: ~8 dummy matmuls keep the tensor
            # engine continuously busy through the DMA preamble so it ramps
            # to 2.4GHz before the first real projection. ----
            if warmup:
                wrm = psP.tile([128, TN], F32, tag="sim", name="wrm")
                for u in range(warmup):
                    nc.tensor.matmul(wrm[:], ones128[:], wsrc[:],
                                     start=(u == 0), stop=(u == warmup - 1),
                                     skip_group_check=True)

            # ---- startup DMA order: first dx superblock ahead of the
            # query/bf16-W streams so the projection pipeline fills ASAP ----
            dx0 = sbx.tile([128, BLOCKS, K_CH // 2, 2 * TN], FP8, tag="dx")
            for hh in range(2):
                nc.sync.dma_start(
                    out=dx0[:, ds(2 * hh, 2)],
                    in_=dT[ds(2 * hh, 2)].rearrange("a p k t -> p a k t"))
            nc.sync.dma_start(out=wt[:], in_=wT[:, :, :])

            # ---- query phase: project + L2-normalize 256 query tokens ----
            qx = qsb.tile([128, K_CH, QTOK], BF16, tag="qx")
            nc.sync.dma_start(out=qx[:], in_=qT[:, :, :])
            qt_ps = psP.tile([128, QTOK], F32, tag="sim")
            for k in range(K_CH):
                nc.tensor.matmul(qt_ps[:], wt[:, k, :], qx[:, k, :],
                                 start=(k == 0), stop=(k == K_CH - 1))
            qt_sb = qsb.tile([128, QTOK], BF16, tag="qtc")
            nc.scalar.activation(qt_sb[:], qt_ps[:], AFT.Copy)
            qsq = qsb.tile([128, QTOK], BF16, tag="qsq")
            nc.vector.tensor_mul(qsq[:], qt_sb[:], qt_sb[:])
            qs2 = psS.tile([128, QTOK], F32, tag="s2")
            nc.tensor.matmul(qs2[:], ones128[:], qsq[:], start=True, stop=True)
            qinv = qsb.tile([128, QTOK], BF16, tag="qinv")
            nc.scalar.activation(qinv[:], qs2[:], AFT.Abs_reciprocal_sqrt)
            nc.vector.tensor_mul(Qn[:], qt_sb[:], qinv[:])

            # ---- doc loop: 8 superblocks of 2048 tokens (1 group) ----
            def emit_proj(g):
                if g == 0:
                    dx = dx0
                else:
                    dx = sbx.tile([128, BLOCKS, K_CH // 2, 2 * TN], FP8,
                                  tag="dx")
                    dma_eng = nc.gpsimd if dx_queue == "gpsimd" else nc.sync
                    for hh in range(dma_split):
                        w_ = BLOCKS // dma_split
                        dma_eng.dma_start(
                            out=dx[:, ds(w_ * hh, w_)],
                            in_=dT[ds(BLOCKS * g + w_ * hh, w_)].rearrange(
                                "a p k t -> p a k t"))
                dts = []
                if pair_tiles:
                    for h in range(2):
                        dt_ps = psD.tile([128, 2 * TN], F32, tag="dt",
                                         name=f"dtp{h}")
                        for jj in range(2):
                            j = 2 * h + jj
                            for kk in range(K_CH // 2):
                                nc.tensor.matmul(
                                    dt_ps[:, ds(jj * TN, TN)],
                                    w8[:, ds(2 * kk, 2), :],
                                    dx[:, j, ds(2 * kk, 2), :],
                                    start=(kk == 0),
                                    stop=(kk == K_CH // 2 - 1),
                                    perf_mode=DR)
                        dt_sb = sbc.tile([128, 2 * TN], BF16, tag="dtc")
                        nc.scalar.activation(dt_sb[:], dt_ps[:], AFT.Copy)
                        dsq = sbq.tile([128, 2 * TN], BF16, tag="dsq")
                        eng = nc.gpsimd if h < dsq_pool else nc.vector
                        eng.tensor_mul(dsq[:], dt_sb[:], dt_sb[:])
                        dts.append((dt_sb[:, 0:TN], dsq[:, 0:TN]))
                        dts.append((dt_sb[:, ds(TN, TN)], dsq[:, ds(TN, TN)]))
                else:
                    dt_ps = [psD.tile([128, TN], F32, tag="dt", name=f"dt{j}")
                             for j in range(BLOCKS)]
                    for j in range(BLOCKS):
                        for kk in range(K_CH // 2):
                            nc.tensor.matmul(
                                dt_ps[j][:], w8[:, ds(2 * kk, 2), :],
                                dx[:, j, kk, :].rearrange(
                                    "p (t k) -> p k t", k=2),
                                start=(kk == 0), stop=(kk == K_CH // 2 - 1),
                                perf_mode=DR)
                    for j in range(BLOCKS):
                        dt_sb = sbc.tile([128, TN], BF16, tag="dtc")
                        nc.scalar.activation(dt_sb[:], dt_ps[j][:], AFT.Copy)
                        dsq = sbq.tile([128, TN], BF16, tag="dsq")
                        eng = nc.gpsimd if j < dsq_pool else nc.vector
                        eng.tensor_mul(dsq[:], dt_sb[:], dt_sb[:])
                        dts.append((dt_sb[:], dsq[:]))
                return dts

            def emit_bands(g, dts):
                sim = psP.tile([128, TN], F32, tag="sim")
                s2p = psS.tile([128, TN], F32, tag="s2")
                for i, (dt_sb, dsq) in enumerate(dts):
                    nc.tensor.matmul(sim[ds(32 * i, 32), :],
                                     Qn[:, ts(g, L_Q)], dt_sb,
                                     start=True, stop=True,
                                     tile_position=(0, 32 * i))
                for i, (dt_sb, dsq) in enumerate(dts):
                    nc.tensor.matmul(s2p[ds(32 * i, 32), :],
                                     ones128[:, 0:32], dsq,
                                     start=True, stop=True,
                                     tile_position=(0, 32 * i))
                inv = sbi.tile([128, TN], BF16, tag="inv")
                nc.scalar.activation(inv[:], s2p[:], AFT.Abs_reciprocal_sqrt,
                                     bias=eps128[:])
                scr = sbs.tile([128, TN], BF16, tag="scr")
                nc.vector.tensor_mul(scr[:], sim[:], inv[:])
                nc.vector.tensor_reduce(
                    mx[:, ds(2 * g, 2)],
                    scr[:].rearrange("p (n d) -> p n d", n=2),
                    axis=mybir.AxisListType.X, op=mybir.AluOpType.max)

            if bands == "insection":
                for g in range(GROUPS):
                    dts = emit_proj(g)
                    emit_bands(g, dts)
            else:
                prev = None
                for g in range(GROUPS):
                    dts = emit_proj(g)
                    if prev is not None:
                        emit_bands(*prev)
                    prev = (g, dts)
                emit_bands(*prev)

            # ---- mean over the 32 queries of each band (via matmul) ----
            mean_ps = psS.tile([BLOCKS, GROUPS * 2], F32, tag="s2")
            nc.tensor.matmul(mean_ps[:], band_ones[:], mx[:],
                             start=True, stop=True)
            nc.vector.tensor_scalar_mul(out_sb[:], mean_ps[:], 1.0 / L_Q)
        nc.sync.dma_start(out=out[:, :], in_=out_sb[:])

    nc.compile()
    return nc


def _shard_inputs(q_hidden, d_hidden, d_input_ids, skiplist, W):
    """Host-side shard + relayout + dtype cast. Returns per-core in_maps."""
    q_hidden = np.asarray(q_hidden, dtype=np.float32)
    d_hidden = np.asarray(d_hidden, dtype=np.float32)
    ids = np.asarray(d_input_ids)
    skip = np.asarray(skiplist)

    # cast first: quarters the bytes the host transposes afterwards
    dh8 = d_hidden.astype(ml_dtypes.float8_e4m3)
    qh16 = q_hidden.astype(ml_dtypes.bfloat16)
    w_t = np.asarray(W, dtype=np.float32).T                   # [768, 128]
    w16 = w_t.astype(ml_dtypes.bfloat16)
    w8 = (w_t * W_SCALE).astype(ml_dtypes.float8_e4m3)

    # zero masked tokens: projections/norms become exactly 0 on device and
    # the biased rsqrt keeps inv finite, so their sims are exactly 0
    masked = (ids == 0) | np.isin(ids, skip)
    dh8[masked] = 0

    wH = np.ascontiguousarray(
        w16.reshape(K_CH, 128, OUT).transpose(1, 0, 2))       # [128, 6, 128]
    w8H = np.ascontiguousarray(
        w8.reshape(K_CH, 128, OUT).transpose(1, 0, 2))
    in_maps = []
    for c in range(N_CORES):
        dh = dh8[c * 64:(c + 1) * 64].reshape(-1, HID)        # [16384, 768]
        qh = qh16[c * GROUPS:(c + 1) * GROUPS].reshape(-1, HID)
        dH = np.ascontiguousarray(
            dh.reshape(D_TILES, TN, K_CH // 2, 2, 128)
            .transpose(0, 4, 2, 1, 3)
            .reshape(D_TILES, 128, K_CH // 2, 2 * TN))
        qH = np.ascontiguousarray(
            qh.reshape(QTOK, K_CH, 128).transpose(2, 1, 0))   # [128, 6, 256]
        in_maps.append({
            "dT": dH,                          # [32, 128, 6, 512] fp8e4m3
            "qT": qH,
            "wT": wH,
            "w8T": w8H,
        })
    return in_maps


_CACHED = {}


def _get_program(key=("default",), **kw):
    if key not in _CACHED:
        _CACHED[key] = _build_program(**kw)
    return _CACHED[key]


def _unpack_out(arr):
    """[4, 16] device tile -> [8 groups, 8 docs]."""
    return np.ascontiguousarray(
        arr.reshape(BLOCKS, GROUPS, 2).transpose(1, 0, 2).reshape(
            GROUPS, N_P))


def kernel(q_hidden, d_hidden, d_input_ids, skiplist, W):
    nc = _get_program(key=("ship",))
    in_maps = _shard_inputs(q_hidden, d_hidden, d_input_ids, skiplist, W)
    res = run_bass_kernel_spmd(nc, in_maps, list(range(N_CORES)))
    outs = [_unpack_out(res.results[c]["out"]) for c in range(N_CORES)]
    return np.concatenate(outs, axis=0)                # (64, 8)
